# revision 19
# baseline (speedup 1.0000x reference)
"""Bass/Trainium2 kernel for nn_MultiHeadAttention (B=4, S=2048, E=512, H=8, dk=dv=8).

Sharding: 8 cores = (batch b, head-half hh).  Core 2b+hh computes causal
attention for batch b over heads [4hh, 4hh+4) for all 2048 queries, applies
its half of the output projection, and returns a partial output transposed
[E, S].  Host sums the two partials per batch, transposes, and adds bo.

Device layout notes:
  - Host feeds query/key/value TRANSPOSED ([E, S]) so projections can use
    them directly as matmul operands (contraction dim on partitions).
  - Projection weights are host-packed into "spread" layouts so projection
    outputs land at partitions {32h+d} (Q/K, row-tiling-ready) or columns
    {9h+d} (V, with a ones column per head at 9h+8 for the softmax
    denominator).
  - Scores are computed transposed ([t, q]) so exp(scores) tiles feed the
    A@V matmul as the moving operand with V as the (cheap) stationary one.
  - 4 heads run concurrently on the PE array: scores via row-tiling
    (tile_position=(32h, 0), K=8) and A@V via col-tiling
    (tile_position=(0, 32h), M=9).
  - Softmax normalization is folded to the end: A@V accumulates
    unnormalized o^T plus a denominator row per head; one strided
    reciprocal + partition-broadcast matmul + multiply normalize before
    the out-proj.

Pipeline schedule (single fused loop, engines kept dense):
  - Per q-chunk c (512 queries): K/V/Q projections for chunk c+1 and the
    out-projection columns of chunk c-1 are interleaved into chunk c's
    t-block loop, so DMA + PSUM-drain latencies hide under attention
    matmuls and the PE ramps to its full 2.4 GHz p-state.
  - scores(tb+1) is emitted before A@V(tb): the PE never waits for the
    Activation engine's exp.
  - All matmul operands are f32r (1 cycle/row for >=256 moving cols; the
    baseline's f32 A@V ran at 4 cycles/row).
  - PSUM: scores ring 2x2 banks, av accumulator double-buffered across
    chunks, 1-bank proj ring, 1-bank rep/outproj ring = exactly 8 banks.
  - Copies run on the (otherwise idle) GpSimd engine; mask-adds,
    reciprocal and normalize-multiplies on Vector; exp owns Activation.
"""

import math

import numpy as np

B, S, E, H = 4, 2048, 512, 8
DK_H = DV_H = 8
NCORES = 8
HPC = H // 2  # heads per core = 4
SCALE = 1.0 / math.sqrt(DK_H)
NEG = -1.0e30
NQC = S // 512  # q chunks of 512
ECH = E // 128  # e chunks of 128

_cache: dict = {}


def _apply_tile_patch():
    """walrus in this image allows only one sync-wait per Drain; split the
    TileContext tail drain's waits across a chain of drains."""
    import concourse.mybir as mybir
    from concourse import tile
    from concourse.vector_clock import ScopedClock

    if getattr(tile.TileContext._drain_and_barrier, "_split_patch", False):
        return

    def _drain_and_barrier_split(self, tick_clock, wait_clock):
        drain_inst = self.nc.sync.drain()
        wait_clock.add_sem_waits(
            drain_inst.ins, ScopedClock({None: tick_clock.global_clock})
        )
        si = drain_inst.ins.sync_info
        if si is not None and si.on_wait and len(si.on_wait) > 1:
            waits = list(si.on_wait)
            si.on_wait = waits[:1]
            for entry in waits[1:]:
                extra = self.nc.sync.drain()
                extra.ins.sync_info = mybir.SyncInfo(on_wait=[entry], on_update=[])
        self.nc.all_engine_barrier()
        assert self.sems is not None
        popped = self.nc._tile_sem_poison_stack.pop()
        assert popped is self._sem_poison
        self.nc.clear_and_free_semaphores(list(self.sems.allocated().values()))
        self.nc.all_engine_barrier()

    _drain_and_barrier_split._split_patch = True
    tile.TileContext._drain_and_barrier = _drain_and_barrier_split


def _split_multi_waits(nc):
    """walrus in this image allows only one sync-wait per instruction;
    move excess waits onto single-wait NOPs inserted just before."""
    import concourse.mybir as mybir

    for blk in nc.m.functions[0].blocks:
        out = []
        for inst in blk.instructions:
            si = getattr(inst, "sync_info", None)
            if si is not None and si.on_wait and len(si.on_wait) > 1:
                waits = list(si.on_wait)
                for i, entry in enumerate(waits[:-1]):
                    out.append(
                        mybir.InstNoOp(
                            name=f"{inst.name}_w{i}",
                            engine=inst.engine,
                            ins=[],
                            outs=[],
                            bass_nofuse=True,
                            sync_info=mybir.SyncInfo(
                                on_wait=[entry], on_update=[]
                            ),
                        )
                    )
                si.on_wait = waits[-1:]
            out.append(inst)
        blk.instructions = out
    return nc


def _build():
    import concourse.bass as bassmod
    import concourse.mybir as mybir
    from concourse import tile

    _apply_tile_patch()
    f32 = mybir.dt.float32
    f32r = mybir.dt.float32r
    bf16 = mybir.dt.bfloat16
    Exp = mybir.ActivationFunctionType.Exp

    def rr(ap):
        return ap.bitcast(f32r)

    nc = bassmod.Bass()
    qT = nc.declare_dram_parameter("qT", [E, S], f32r, isOutput=False)
    kT = nc.declare_dram_parameter("kT", [E, S], f32r, isOutput=False)
    vT = nc.declare_dram_parameter("vT", [E, S], f32r, isOutput=False)
    wq = nc.declare_dram_parameter("wq", [E, 128], f32r, isOutput=False)
    wk = nc.declare_dram_parameter("wk", [E, 128], f32r, isOutput=False)
    wv = nc.declare_dram_parameter("wv", [E, HPC * 9], f32r, isOutput=False)
    wo = nc.declare_dram_parameter("wo", [128, E], f32r, isOutput=False)
    msk = nc.declare_dram_parameter("msk", [128, 2 * 128], f32, isOutput=False)
    out = nc.declare_dram_parameter("out", [E, S], f32, isOutput=True)

    with tile.TileContext(nc) as tc:
        with (
            tc.tile_pool(name="singles", bufs=1) as singles,
            tc.tile_pool(name="loads", bufs=6) as loads,
            tc.tile_pool(name="abuf", bufs=4) as abuf,
            tc.tile_pool(name="outs", bufs=4) as outs,
            tc.tile_pool(name="ps_sc", bufs=2, space="PSUM") as ps_sc,
            tc.tile_pool(name="ps_av", bufs=2, space="PSUM") as ps_av,
            tc.tile_pool(name="ps_opj", bufs=1, space="PSUM") as ps_opj,
            tc.tile_pool(name="ps_pj", bufs=1, space="PSUM") as ps_pj,
        ):
            # ---- resident tensors -------------------------------------
            wq_sb = singles.tile([128, ECH, 128], f32r, tag="wq")
            wk_sb = singles.tile([128, ECH, 128], f32r, tag="wk")
            wv_sb = singles.tile([128, ECH, HPC * 9], f32r, tag="wv")
            wo_sb = singles.tile([128, ECH, 128], f32r, tag="wo")
            msk_sb = singles.tile([128, 2, 128], f32, tag="msk")
            # spread dispatch across engine queues: each dma_start costs
            # ~1us of queue time, so serializing them on sync delays the
            # first projection matmul
            nc.sync.dma_start(out=wk_sb, in_=wk.rearrange("(c p) m -> p c m", p=128))
            nc.gpsimd.dma_start(out=wq_sb, in_=wq.rearrange("(c p) m -> p c m", p=128))
            nc.gpsimd.dma_start(out=wv_sb, in_=wv.rearrange("(c p) m -> p c m", p=128))
            nc.gpsimd.dma_start(out=wo_sb, in_=wo.rearrange("p (c m) -> p c m", c=ECH))
            nc.gpsimd.dma_start(out=msk_sb, in_=msk.rearrange("p (g n) -> p g n", g=2))

            KTs = singles.tile([128, S], f32r, tag="KTs")
            QTs = singles.tile([128, S], f32r, tag="QTs")
            Vsb = singles.tile([128, S // 128, HPC, 9], bf16, tag="Vsb")
            onorm = singles.tile([128, S], f32r, tag="onorm")
            recip = singles.tile([128, S], f32, tag="recip")
            recip_r = singles.tile([128, S], bf16, tag="recip_r")
            rrep = singles.tile([128, 512], f32, tag="rrep")

            ones9 = singles.tile([128, 9], bf16, tag="ones9")
            nc.vector.memset(ones9, 1.0)

            # zero backgrounds (stale SBUF must not reach the PE as NaN)
            nc.vector.memset(onorm.bitcast(f32), 0.0)
            # ones columns for the denominator accumulation
            nc.vector.memset(Vsb[:, :, :, 0:1], 1.0)

            qTr = qT.rearrange("(c p) m -> p c m", p=128)
            kTr = kT.rearrange("(c p) m -> p c m", p=128)
            vTr = vT.rearrange("(c p) m -> p c m", p=128)

            ld_tiles = {}

            def emit_load(nm, c, eng=None):
                cs = slice(c * 512, (c + 1) * 512)
                src = {"k": kTr, "q": qTr, "v": vTr}[nm]
                t = loads.tile([128, ECH, 512], f32r, tag="ld",
                               name=f"ld_{nm}{c}")
                (eng or nc.sync).dma_start(out=t, in_=src[:, :, cs])
                ld_tiles[(nm, c)] = t

            def emit_loads(c):
                for nm in ("k", "q", "v"):
                    emit_load(nm, c)

            def emit_proj_k(c, pool, ptag):
                cs = slice(c * 512, (c + 1) * 512)
                ld = ld_tiles.pop(("k", c))
                ps = pool.tile([128, 512], f32, tag=ptag)
                for e in range(ECH):
                    nc.tensor.matmul(
                        ps, wk_sb[:, e, :], ld[:, e, :],
                        start=(e == 0), stop=(e == ECH - 1),
                    )
                nc.vector.tensor_copy(KTs[:, cs], ps)

            def emit_proj_q(c, pool, ptag):
                cs = slice(c * 512, (c + 1) * 512)
                ld = ld_tiles.pop(("q", c))
                ps = pool.tile([128, 512], f32, tag=ptag)
                for e in range(ECH):
                    nc.tensor.matmul(
                        ps, wq_sb[:, e, :], ld[:, e, :],
                        start=(e == 0), stop=(e == ECH - 1),
                    )
                nc.vector.tensor_copy(QTs[:, cs], ps)

            def emit_proj_v(c, pool, ptag):
                ld = ld_tiles.pop(("v", c))
                ps = pool.tile([128, 4, HPC * 9], f32, tag=ptag)
                for sb in range(4):  # 128-row t sub-blocks within the chunk
                    for e in range(ECH):
                        nc.tensor.matmul(
                            ps[:, sb, :],
                            ld[:, e, sb * 128:(sb + 1) * 128],
                            wv_sb[:, e, :],
                            start=(e == 0), stop=(e == ECH - 1),
                        )
                src = ps.rearrange("p s (h n) -> p s h n", n=9)[:, :, :, 1:9]
                nc.vector.tensor_copy(Vsb[:, 4 * c:4 * c + 4, :, 1:9], src)

            av_tiles = {}
            sc_live = {}

            def emit_scores(c, tb):
                cs = slice(c * 512, (c + 1) * 512)
                d = 128 * tb - 512 * c  # diagonal offset within the chunk
                vstart = max(d, 0)
                # f32r moving <256 cols runs at 1/4 rate; don't over-trim
                mmstart = min(vstart, 256)
                scs = [
                    ps_sc.tile([128, 2, 512], f32, tag="sc",
                               name=f"sc{c}_{tb}_0"),
                    ps_sc.tile([128, 2, 512], f32, tag="sc",
                               name=f"sc{c}_{tb}_1"),
                ]
                ats = [
                    abuf.tile([128, 2, 512], bf16, tag="a",
                              name=f"a{c}_{tb}_0"),
                    abuf.tile([128, 2, 512], bf16, tag="a",
                              name=f"a{c}_{tb}_1"),
                ]
                for h in range(HPC):
                    g, j = divmod(h, 2)
                    nc.tensor.matmul(
                        scs[g][:, j, mmstart:512],
                        KTs[32 * h:32 * h + 8, tb * 128:(tb + 1) * 128],
                        QTs[32 * h:32 * h + 8, cs][:, mmstart:512],
                        start=True, stop=True,
                        tile_position=(32 * h, 0),
                    )
                for g in range(2):
                    if d >= 0:
                        nc.vector.tensor_add(
                            scs[g][:, :, d:d + 128],
                            scs[g][:, :, d:d + 128],
                            msk_sb,
                        )
                    nc.scalar.activation(
                        ats[g][:, :, vstart:512], scs[g][:, :, vstart:512],
                        Exp, scale=SCALE,
                    )
                sc_live[(c, tb)] = (ats, vstart)

            def emit_av(c, tb):
                ats, vstart = sc_live.pop((c, tb))
                ntb = 4 * (c + 1)
                av = av_tiles[c]
                for h in range(HPC):
                    g, j = divmod(h, 2)
                    nc.tensor.matmul(
                        av[32 * h:32 * h + 9, vstart:512],
                        Vsb[:, tb, h, :],
                        ats[g][:, j, vstart:512],
                        start=(tb == 0), stop=(tb == ntb - 1),
                        tile_position=(0, 32 * h),
                    )

            def emit_norm(c):
                cs = slice(c * 512, (c + 1) * 512)
                av = av_tiles.pop(c)
                # full-tile ops: only the 4 denominator partitions are
                # ever read downstream
                nc.vector.reciprocal(recip[:, cs], av)
                nc.vector.tensor_copy(recip_r[:, cs], recip[:, cs])
                rep_ps = ps_opj.tile([128, 512], f32, tag="oj")
                for h in range(HPC):
                    nc.tensor.matmul(
                        rep_ps[32 * h:32 * h + 9, :],
                        ones9[32 * h:32 * h + 1, :],
                        recip_r[32 * h:32 * h + 1, cs],
                        start=True, stop=True,
                        tile_position=(32 * h, 32 * h),
                    )
                nc.vector.tensor_copy(rrep, rep_ps)
                for h in range(HPC):
                    nc.vector.tensor_mul(
                        onorm[32 * h:32 * h + 9, cs],
                        av[32 * h:32 * h + 9, :],
                        rrep[32 * h:32 * h + 9, :],
                    )

            def emit_opj(c, e, pool, ptag):
                cs = slice(c * 512, (c + 1) * 512)
                f_ps = pool.tile([128, 512], f32, tag=ptag)
                nc.tensor.matmul(
                    f_ps, wo_sb[:, e, :], onorm[:, cs],
                    start=True, stop=True,
                )
                fsb = outs.tile([128, 512], f32, tag="f")
                nc.vector.tensor_copy(fsb, f_ps)
                nc.sync.dma_start(out=out[e * 128:(e + 1) * 128, cs], in_=fsb)

            # ---- prologue ---------------------------------------------
            # k/q first so the K/Q projections start as soon as possible
            emit_load("k", 0)
            emit_load("q", 0, nc.scalar)
            emit_load("v", 0, nc.scalar)
            emit_proj_k(0, ps_pj, "pj")
            emit_proj_q(0, ps_opj, "oj")
            emit_proj_v(0, ps_pj, "pj")
            emit_loads(1)
            emit_loads(2)

            # ---- flattened chunk pipeline -----------------------------
            # one software-pipelined stream across ALL t-blocks: scores(i+1)
            # is emitted before A@V(i), including across chunk boundaries,
            # so the PE never drains while exp catches up.  extras (next
            # chunk's projections, previous chunk's out-projection) slot
            # between scores and A@V.
            units = [(c, tb) for c in range(NQC) for tb in range(4 * (c + 1))]
            extras = {}
            extras[(0, 1)] = [lambda: emit_proj_k(1, ps_pj, "pj")]
            extras[(0, 2)] = [lambda: emit_proj_v(1, ps_pj, "pj")]
            extras[(0, 3)] = [lambda: emit_proj_q(1, ps_pj, "pj")]
            for c in range(1, NQC):
                for i in range(4):
                    extras.setdefault((c, 2 + i), []).append(
                        lambda cc=c - 1, e=i: emit_opj(cc, e, ps_opj, "oj")
                    )
                if c < NQC - 1:
                    extras.setdefault((c, 5), []).append(
                        lambda cc=c + 1: emit_proj_k(cc, ps_pj, "pj")
                    )
                    extras.setdefault((c, 6), []).append(
                        lambda cc=c + 1: emit_proj_v(cc, ps_pj, "pj")
                    )
                    extras.setdefault((c, 7), []).append(
                        lambda cc=c + 1: emit_proj_q(cc, ps_pj, "pj")
                    )

            for i, (c, tb) in enumerate(units):
                if tb == 0:
                    av_tiles[c] = ps_av.tile([128, 512], f32, tag="av",
                                             name=f"av{c}")
                if (c, tb) == (1, 0):
                    emit_loads(3)
                emit_scores(c, tb)
                for fn in extras.get((c, tb), []):
                    fn()
                if i >= 1:
                    pc, ptb = units[i - 1]
                    emit_av(pc, ptb)
                    if ptb == 4 * (pc + 1) - 1:
                        emit_norm(pc)
            emit_av(NQC - 1, 4 * NQC - 1)
            # final chunk: split normalize + out-proj into two q-halves so
            # the serial recip/mul chain overlaps the out-proj matmuls, and
            # use the now-idle Activation engine for PSUM drains
            c3 = NQC - 1
            av3 = av_tiles.pop(c3)
            for half in range(2):
                hs = slice(half * 256, (half + 1) * 256)
                gcs = slice(c3 * 512 + half * 256, c3 * 512 + (half + 1) * 256)
                nc.vector.reciprocal(recip[:, gcs], av3[:, hs])
                nc.vector.tensor_copy(recip_r[:, gcs], recip[:, gcs])
                rep_ps = ps_opj.tile([128, 256], f32, tag="oj",
                                     name=f"rep3_{half}")
                for h in range(HPC):
                    nc.tensor.matmul(
                        rep_ps[32 * h:32 * h + 9, :],
                        ones9[32 * h:32 * h + 1, :],
                        recip_r[32 * h:32 * h + 1, gcs],
                        start=True, stop=True,
                        tile_position=(32 * h, 32 * h),
                    )
                nc.vector.tensor_copy(rrep[:, hs], rep_ps)
                for h in range(HPC):
                    nc.vector.tensor_mul(
                        onorm[32 * h:32 * h + 9, gcs],
                        av3[32 * h:32 * h + 9, hs],
                        rrep[32 * h:32 * h + 9, hs],
                    )
                for e in range(ECH):
                    pool, ptag = (ps_pj, "pj") if e % 2 else (ps_opj, "oj")
                    f_ps = pool.tile([128, 256], f32, tag=ptag,
                                     name=f"f3_{half}_{e}")
                    nc.tensor.matmul(
                        f_ps, wo_sb[:, e, :], onorm[:, gcs],
                        start=True, stop=True,
                    )
                    fsb = outs.tile([128, 256], f32, tag="f",
                                    name=f"fsb3_{half}_{e}")
                    nc.scalar.copy(fsb, f_ps)
                    nc.gpsimd.dma_start(
                        out=out[e * 128:(e + 1) * 128, gcs], in_=fsb
                    )

    _split_multi_waits(nc)
    return nc


def _prep_inputs(query, key, value, Wq, Wk, Wv, Wo):
    """Build the 8 per-core input maps (host-side sharding/layout)."""
    qTs = [np.ascontiguousarray(query[b].T) for b in range(B)]
    kTs = [np.ascontiguousarray(key[b].T) for b in range(B)]
    vTs = [np.ascontiguousarray(value[b].T) for b in range(B)]

    mask = np.where(
        np.arange(128)[:, None] <= np.arange(128)[None, :], 0.0, NEG
    ).astype(np.float32)
    msk2 = np.ascontiguousarray(np.tile(mask, (1, 2)))

    in_maps = []
    for core in range(NCORES):
        b, hh = divmod(core, 2)
        wq_p = np.zeros((E, 128), np.float32)
        wk_p = np.zeros((E, 128), np.float32)
        wv_p = np.zeros((E, HPC * 9), np.float32)
        wo_p = np.zeros((128, E), np.float32)
        for h in range(HPC):
            g = 4 * hh + h
            wq_p[:, 32 * h:32 * h + 8] = Wq[g]
            wk_p[:, 32 * h:32 * h + 8] = Wk[g]
            wv_p[:, 9 * h + 1:9 * h + 9] = Wv[g]
            wo_p[32 * h + 1:32 * h + 9, :] = Wo[8 * g:8 * g + 8, :]
        in_maps.append(
            {
                "qT": qTs[b], "kT": kTs[b], "vT": vTs[b],
                "wq": wq_p, "wk": wk_p, "wv": wv_p, "wo": wo_p,
                "msk": msk2,
            }
        )
    return in_maps


def _reference_numpy(query, key, value, padding_mask, decoder_mask,
                     Wq, Wk, Wv, Wo, bo):
    """Fallback (non-default masks): plain numpy replica of the reference."""
    q = np.einsum("bse,hed->bhsd", query, Wq)
    k = np.einsum("bse,hed->bhsd", key, Wk)
    v = np.einsum("bse,hed->bhsd", value, Wv)
    s = np.einsum("bhsd,bhtd->bhst", q, k)
    if decoder_mask:
        tril = np.tril(s)
        s = np.where(tril == 0.0, -np.inf, s)
    s = np.where(padding_mask[:, None, :, :], s, -np.inf)
    s = s / np.sqrt(np.float32(DK_H))
    m = np.max(s, axis=-1, keepdims=True)
    e = np.exp(s - m)
    a = e / np.sum(e, axis=-1, keepdims=True)
    o = np.einsum("bhst,bhtd->bhsd", a, v)
    o = o.transpose(0, 2, 1, 3).reshape(o.shape[0], o.shape[2], H * DV_H)
    return (o @ Wo + bo).astype(np.float32)


def kernel(query, key, value, padding_mask, decoder_mask, Wq, Wk, Wv, Wo, bo,
           **run_kwargs):
    query = np.asarray(query, np.float32)
    key = np.asarray(key, np.float32)
    value = np.asarray(value, np.float32)
    Wq = np.asarray(Wq, np.float32)
    Wk = np.asarray(Wk, np.float32)
    Wv = np.asarray(Wv, np.float32)
    Wo = np.asarray(Wo, np.float32)
    bo = np.asarray(bo, np.float32)
    pm = np.asarray(padding_mask)
    dm = int(np.asarray(decoder_mask))

    if not bool(pm.all()) or not dm:
        return _reference_numpy(
            query, key, value, pm.astype(bool), dm, Wq, Wk, Wv, Wo, bo
        )

    from concourse.bass_utils import run_bass_kernel_spmd

    if "nc" not in _cache:
        _cache["nc"] = _build()
    nc = _cache["nc"]

    in_maps = _prep_inputs(query, key, value, Wq, Wk, Wv, Wo)
    res = run_bass_kernel_spmd(nc, in_maps, list(range(NCORES)), **run_kwargs)

    outp = np.empty((B, S, E), np.float32)
    for b in range(B):
        fT = res.results[2 * b]["out"] + res.results[2 * b + 1]["out"]
        outp[b] = fT.T + bo
    if run_kwargs:
        kernel.last_result = res
    return outp
